# revision 19
# baseline (speedup 1.0000x reference)
"""EM-routing (DigitCaps) kernel for 8 trn2 NeuronCores.

Sharding: data-parallel over batch B=16 -> 2 samples per core, no collectives.

Per-core algorithm (N=16384, C=10, D=16, 3 EM iterations, output = miu):
  - votes stored in SBUF f32 N-layout vv[p, n2, 0:160]=v, [160:320]=v^2,
    [320]=1.0, with n = p*128 + n2  (one sample at a time: 160.5KB/partition).
  - All n-reductions (M1=sum_n r*v, M2=sum_n r*v^2, R=sum_n r) are PE matmuls
    with the per-iteration routing weights r as the stationary operand
    (plain fp32 matmuls: exact, 4 cyc/row). The matmul
    produces the full 10x160 cross matrix; the needed diagonal c-blocks are
    extracted via a DRAM round-trip (diag AP [[176,10],[1,16]]).
  - sigma via moments: sigma = M2/R - miu^2*(2-S) + eps, S = R/(R+eps).
  - log-p d-contraction: q[n,c] = A2_c + sum_d v*g - sum_d v^2*s2 with
    g=miu/sigma, s2=1/(2*sigma) broadcast along partitions; computed by DVE
    tensor ops + segmented reduces.
  - The reference's per-n max-shift is replaced by the constant upper bound
    max_cd(h), h=-0.5*log(sigma) (difference enters only through eps, ~1e-21).
"""

import numpy as np

B, N, C, D = 16, 16384, 10, 16
CD = C * D
NB = 2          # samples per core
NCORES = 8
P = 128         # partitions
NCH = N // P    # 128 chunks; n = p*128 + n2
EPS = 1e-9
LN10 = float(np.log(10.0))
SLAB = 8        # chunks per z-slab
NSLAB = NCH // SLAB

_CACHE = {}


def _build():
    from contextlib import ExitStack
    import concourse.bass as bass
    import concourse.bacc as bacc
    import concourse.mybir as mybir
    import concourse.tile as tile
    import concourse.bass_isa as bass_isa

    f32 = mybir.dt.float32
    f32r = mybir.dt.float32r
    AX = mybir.AxisListType
    OP = mybir.AluOpType
    ACTF = mybir.ActivationFunctionType

    nc = bacc.Bacc("TRN2")
    votes_d = nc.dram_tensor("votes", [NB, N, C, D], f32, kind="ExternalInput")
    act_d = nc.dram_tensor("activation", [NB, N, 1], f32, kind="ExternalInput")
    out_d = nc.dram_tensor("miu_out", [NB, C, D], f32, kind="ExternalOutput")

    def bfree(t_ap, nrep):
        # read-broadcast an SBUF AP along a new middle free dim (step 0)
        return bass.AP(
            tensor=t_ap.tensor,
            offset=t_ap.offset,
            ap=[t_ap.ap[0], [0, nrep]] + list(t_ap.ap[1:]),
        )

    with ExitStack() as ctx:
        tc = ctx.enter_context(tile.TileContext(nc))
        big = ctx.enter_context(tc.tile_pool(name="big", bufs=1))
        zpool = ctx.enter_context(tc.tile_pool(name="z", bufs=3))
        qpool = ctx.enter_context(tc.tile_pool(name="q", bufs=1))
        rpool = ctx.enter_context(tc.tile_pool(name="r", bufs=1))
        small = ctx.enter_context(tc.tile_pool(name="small", bufs=1))
        tiny = ctx.enter_context(tc.tile_pool(name="tiny", bufs=1))
        psum = ctx.enter_context(tc.tile_pool(name="psum", bufs=2, space="PSUM"))

        # persistent across the whole kernel
        vv = big.tile([P, NCH, 321], f32)          # [v | v^2 | ones]
        a_t = big.tile([P, NCH], f32)              # activation
        act_b = big.tile([P, C], f32)              # act_out bcast (free c)

        for s in range(NB):
            # ---------------- load sample s ----------------
            if s > 0:
                nc.all_engine_barrier()
            vsrc = votes_d[s].rearrange("(p n2) c d -> p n2 (c d)", p=P)
            for g in range(16):
                sl = slice(g * SLAB, (g + 1) * SLAB)
                nc.gpsimd.dma_start(out=vv[:, sl, 0:160], in_=vsrc[:, sl, :])
                nc.scalar.activation(
                    out=vv[:, sl, 161:321], in_=vv[:, sl, 0:160], func=ACTF.Square
                )
                nc.scalar.activation(
                    out=vv[:, sl, 160:161], in_=vv[:, sl, 0:1],
                    func=ACTF.Copy, bias=1.0, scale=0.0,
                )
            nc.gpsimd.dma_start(
                out=a_t[:, :], in_=act_d[s].rearrange("(p n2) one -> p (n2 one)", p=P)
            )

            r_t = rpool.tile([P, NCH, C], f32, tag="r")
            q_t = qpool.tile([P, NCH, C], f32)

            miu_diag = None
            for it in range(3):
                # ---------------- r computation ----------------
                if it == 0:
                    t0 = small.tile([P, NCH], f32, tag="sm0")
                    nc.vector.tensor_scalar_add(t0, a_t, EPS)
                    t1 = small.tile([P, NCH], f32, tag="sm1")
                    nc.vector.reciprocal(t1, t0)
                    t2 = small.tile([P, NCH], f32, tag="sm2")
                    nc.vector.tensor_tensor(out=t2, in0=a_t, in1=t1, op=OP.mult)
                    # r0 = a/(C*(a+eps)) broadcast over c (on ACT so the
                    # first matmul's waits collapse to {ACT, DMA})
                    nc.scalar.activation(
                        r_t[:, :, :],
                        bass.AP(
                            tensor=t2.tensor,
                            offset=t2.offset,
                            ap=[t2.ap[0], t2.ap[1], [0, C]],
                        ),
                        func=ACTF.Copy,
                        scale=1.0 / C,
                    )
                    # canary: dummy matmul consuming an ACT-written tile so the
                    # real matmuls' PE-side waits collapse to one semaphore
                    cn = tiny.tile([1, 1], f32, tag="cn")
                    nc.scalar.activation(out=cn, in_=a_t[0:1, 0:1], func=ACTF.Copy)
                    psc = psum.tile([1, 1], f32, tag="psc")
                    nc.tensor.matmul(psc, cn, cn, start=True, stop=True)
                else:
                    # params from miu_diag/sig_diag (single-partition [1,160])
                    inv_sig = tiny.tile([1, CD], f32, tag="pinv")
                    nc.vector.reciprocal(inv_sig, sig_diag)
                    g_f = tiny.tile([1, CD], f32, tag="pg")
                    nc.vector.tensor_tensor(out=g_f, in0=miu_diag, in1=inv_sig, op=OP.mult)
                    s2_f = tiny.tile([1, CD], f32, tag="ps2")
                    nc.vector.tensor_scalar_mul(s2_f, inv_sig, 0.5)
                    ls_f = tiny.tile([1, CD], f32, tag="pls")
                    nc.scalar.activation(out=ls_f, in_=sig_diag, func=ACTF.Ln)
                    # u = log(sig) + miu^2/sig ; A = -0.5 * sum_d u per c
                    w_f = tiny.tile([1, CD], f32, tag="pw")
                    nc.vector.tensor_tensor(out=w_f, in0=miu_diag, in1=g_f, op=OP.mult)
                    u_f = tiny.tile([1, CD], f32, tag="pu")
                    nc.vector.tensor_tensor(out=u_f, in0=ls_f, in1=w_f, op=OP.add)
                    Ac = tiny.tile([1, C], f32, tag="pAc")
                    nc.vector.tensor_reduce(
                        Ac, u_f.rearrange("one (c d) -> one c d", c=C), axis=AX.X, op=OP.add
                    )
                    # m' = max_cd h = -0.5*min_cd log(sig)
                    lmin = tiny.tile([1, 1], f32, tag="plm")
                    nc.vector.tensor_reduce(lmin, ls_f, axis=AX.X, op=OP.min)
                    # A2_c = -0.5*Ac + D*(ln10 + 0.5*lmin)
                    bias_t = tiny.tile([1, 1], f32, tag="pbi")
                    nc.vector.tensor_scalar(
                        bias_t, lmin, 0.5 * float(D), float(D) * LN10, OP.mult, OP.add
                    )
                    A2 = tiny.tile([1, C], f32, tag="pA2")
                    nc.vector.tensor_scalar(A2, Ac, -0.5, bias_t, OP.mult, OP.add)
                    A2_128 = small.tile([P, C], f32, tag="a2b")
                    nc.gpsimd.partition_broadcast(A2_128, A2)
                    g_b = small.tile([P, CD], f32, tag="gb")
                    nc.gpsimd.partition_broadcast(g_b, g_f)
                    s2_b = small.tile([P, CD], f32, tag="s2b")
                    nc.gpsimd.partition_broadcast(s2_b, s2_f)

                    # ---------------- q volume: z1,z2 + segment reduce ------
                    g_b3 = g_b.rearrange("p (c d) -> p c d", c=C)
                    s2_b3 = s2_b.rearrange("p (c d) -> p c d", c=C)
                    for sb in range(NSLAB):
                        sl = slice(sb * SLAB, (sb + 1) * SLAB)
                        z1 = zpool.tile([P, SLAB, C, D], f32, tag="z")
                        nc.vector.tensor_tensor(
                            out=z1,
                            in0=vv[:, sl, 0:160].rearrange("p s (c d) -> p s c d", c=C),
                            in1=bfree(g_b3, SLAB),
                            op=OP.mult,
                        )
                        z2 = zpool.tile([P, SLAB, C, D], f32, tag="z")
                        nc.gpsimd.tensor_tensor(
                            out=z2,
                            in0=vv[:, sl, 161:321].rearrange("p s (c d) -> p s c d", c=C),
                            in1=bfree(s2_b3, SLAB),
                            op=OP.mult,
                        )
                        q1 = zpool.tile([P, SLAB, C], f32, tag="qs")
                        nc.vector.tensor_reduce(q1, z1, axis=AX.X, op=OP.add)
                        q2 = zpool.tile([P, SLAB, C], f32, tag="qs")
                        nc.vector.tensor_reduce(q2, z2, axis=AX.X, op=OP.add)
                        nc.vector.tensor_tensor(
                            out=q_t[:, sl, :], in0=q1, in1=q2, op=OP.subtract
                        )
                    # q += A2 ; p = exp(q) (in-place chain on q_t)
                    nc.vector.tensor_tensor(
                        out=q_t[:, :, :],
                        in0=q_t[:, :, :],
                        in1=bfree(A2_128, NCH),
                        op=OP.add,
                    )
                    nc.scalar.activation(out=q_t[:, :, :], in_=q_t[:, :, :], func=ACTF.Exp)
                    ap_t = q_t
                    nc.vector.tensor_tensor(
                        out=ap_t[:, :, :], in0=ap_t[:, :, :], in1=bfree(act_b, NCH), op=OP.mult
                    )
                    sc = small.tile([P, NCH], f32, tag="sm0")
                    nc.vector.tensor_reduce(sc, ap_t, axis=AX.X, op=OP.add)
                    d1 = small.tile([P, NCH], f32, tag="sm1")
                    nc.vector.tensor_scalar_add(d1, sc, EPS)
                    d2 = small.tile([P, NCH], f32, tag="sm2")
                    nc.vector.reciprocal(d2, d1)
                    fac1 = small.tile([P, NCH], f32, tag="sm3")
                    nc.vector.tensor_tensor(out=fac1, in0=d2, in1=a_t, op=OP.mult)
                    r2_t = q_t
                    nc.vector.tensor_tensor(
                        out=r2_t[:, :, :],
                        in0=ap_t[:, :, :],
                        in1=bass.AP(
                            tensor=fac1.tensor,
                            offset=fac1.offset,
                            ap=[fac1.ap[0], fac1.ap[1], [0, C]],
                        ),
                        op=OP.mult,
                    )
                    sc2 = small.tile([P, NCH], f32, tag="sm0")
                    nc.vector.tensor_reduce(sc2, r2_t[:, :, :], axis=AX.X, op=OP.add)
                    d3 = small.tile([P, NCH], f32, tag="sm1")
                    nc.vector.tensor_scalar_add(d3, sc2, EPS)
                    d4 = small.tile([P, NCH], f32, tag="sm2")
                    nc.vector.reciprocal(d4, d3)
                    nc.vector.tensor_tensor(
                        out=r_t[:, :, :],
                        in0=r2_t[:, :, :],
                        in1=bass.AP(
                            tensor=d4.tensor,
                            offset=d4.offset,
                            ap=[d4.ap[0], d4.ap[1], [0, C]],
                        ),
                        op=OP.mult,
                    )

                # ---------------- PE: M1|M2|R matmuls ----------------
                FW = 321 if it < 2 else 161
                mm = psum.tile([10, 321], f32, tag="mm")
                for n2 in range(NCH):
                    nc.tensor.matmul(
                        mm[:, 0:FW],
                        r_t[:, n2, :],
                        vv[:, n2, 0:FW],
                        start=(n2 == 0),
                        stop=(n2 == NCH - 1),
                    )

                # ---------------- stats (full 10x160 cross matrices) --------
                Rse = tiny.tile([10, 1], f32, tag="cRs")
                nc.vector.tensor_scalar_add(Rse, mm[:, 160:161], EPS)
                invR = tiny.tile([10, 1], f32, tag="cIR")
                nc.vector.reciprocal(invR, Rse)
                miu_full = small.tile([10, CD], f32, tag="miuf")
                nc.vector.tensor_scalar(miu_full, mm[:, 0:160], invR, None, OP.mult)

                if it < 2:
                    m2n = tiny.tile([10, CD], f32, tag="cm2")
                    nc.vector.tensor_scalar(m2n, mm[:, 161:321], invR, None, OP.mult)
                    S_t = tiny.tile([10, 1], f32, tag="cS")
                    nc.vector.tensor_scalar(S_t, mm[:, 160:161], invR, None, OP.mult)
                    c2_t = tiny.tile([10, 1], f32, tag="cc2")
                    nc.vector.tensor_scalar(c2_t, S_t, -1.0, 2.0, OP.mult, OP.add)
                    mmu = tiny.tile([10, CD], f32, tag="cmu")
                    nc.vector.tensor_tensor(out=mmu, in0=miu_full, in1=miu_full, op=OP.mult)
                    nc.vector.tensor_scalar(mmu, mmu, c2_t, None, OP.mult)
                    sig_full = tiny.tile([10, CD], f32, tag="csg")
                    nc.vector.tensor_scalar(sig_full, m2n, EPS, None, OP.add)
                    nc.vector.tensor_tensor(out=sig_full, in0=sig_full, in1=mmu, op=OP.subtract)

                    miu_diag = tiny.tile([1, CD], f32, tag="dgm")
                    sig_diag = tiny.tile([1, CD], f32, tag="dgs")
                    if it == 0:
                        # r0 is c-uniform: every row of the cross matrix is
                        # the diagonal -> plain same-partition copies
                        nc.vector.tensor_copy(miu_diag, miu_full[0:1, :])
                        nc.vector.tensor_copy(sig_diag, sig_full[0:1, :])
                    else:
                        # diag extraction via DRAM round-trip
                        scr = nc.dram_tensor(f"scr_{s}_{it}", [4096], f32, kind="Internal")[:]
                        nc.gpsimd.dma_start(out=scr[0:1600], in_=miu_full)
                        nc.gpsimd.dma_start(out=scr[1600:3200], in_=sig_full)
                        diag_miu_ap = bass.AP(
                            tensor=scr.tensor,
                            offset=scr.offset,
                            ap=[[0, 1], [176, 10], [1, 16]],
                        )
                        diag_sig_ap = bass.AP(
                            tensor=scr.tensor,
                            offset=scr.offset + 1600,
                            ap=[[0, 1], [176, 10], [1, 16]],
                        )
                        nc.gpsimd.dma_start(out=miu_diag, in_=diag_miu_ap)
                        nc.gpsimd.dma_start(out=sig_diag, in_=diag_sig_ap)

                    # act_out for next iter
                    if it == 0:
                        nc.vector.memset(act_b, 1.0 / C)
                    else:
                        nc.gpsimd.dma_start(out=scr[3200:3210], in_=Rse)
                        Rf = tiny.tile([1, C], f32, tag="pRf")
                        nc.gpsimd.dma_start(out=Rf, in_=scr[3200:3210])
                        mx = tiny.tile([1, 1], f32, tag="pmx")
                        nc.vector.tensor_reduce(mx, Rf, axis=AX.X, op=OP.max)
                        sh = tiny.tile([1, C], f32, tag="psh")
                        nc.vector.tensor_scalar(sh, Rf, mx, None, OP.subtract)
                        ex = tiny.tile([1, C], f32, tag="pex")
                        nc.scalar.activation(out=ex, in_=sh, func=ACTF.Exp)
                        sm = tiny.tile([1, 1], f32, tag="psm")
                        nc.vector.tensor_reduce(sm, ex, axis=AX.X, op=OP.add)
                        smr = tiny.tile([1, 1], f32, tag="psr")
                        nc.vector.reciprocal(smr, sm)
                        ao = tiny.tile([1, C], f32, tag="pao")
                        nc.vector.tensor_scalar(ao, ex, smr, None, OP.mult)
                        nc.gpsimd.partition_broadcast(act_b, ao)
                else:
                    # final: extract diag of miu_full -> output
                    scr = nc.dram_tensor(f"scr_{s}_{it}", [4096], f32, kind="Internal")[:]
                    nc.gpsimd.dma_start(out=scr[0:1600], in_=miu_full)
                    out_diag = bass.AP(
                        tensor=scr.tensor,
                        offset=scr.offset,
                        ap=[[0, 1], [176, 10], [1, 16]],
                    )
                    fin = tiny.tile([1, CD], f32, tag="dgm")
                    nc.gpsimd.dma_start(out=fin, in_=out_diag)
                    nc.gpsimd.dma_start(
                        out=out_d[s].rearrange("c d -> (c d)"), in_=fin
                    )

    nc.compile()
    return nc


def kernel(votes, activation, beta_v, beta_a):
    from concourse.bass_utils import run_bass_kernel_spmd

    if "nc" not in _CACHE:
        _CACHE["nc"] = _build()
    nc = _CACHE["nc"]

    votes = np.ascontiguousarray(votes, dtype=np.float32)
    activation = np.ascontiguousarray(activation, dtype=np.float32)
    in_maps = [
        {
            "votes": votes[i * NB : (i + 1) * NB],
            "activation": activation[i * NB : (i + 1) * NB],
        }
        for i in range(NCORES)
    ]
    res = run_bass_kernel_spmd(nc, in_maps, core_ids=list(range(NCORES)))
    out = np.concatenate([res.results[i]["miu_out"] for i in range(NCORES)], axis=0)
    return out.reshape(B, 1, C, D).astype(np.float32)


if __name__ == "__main__":
    _build()
    print("build OK")


# revision 20
# speedup vs baseline: 1.1341x; 1.1341x over previous
"""EM-routing (DigitCaps) kernel for 8 trn2 NeuronCores.

Sharding: data-parallel over batch B=16 -> 2 samples per core, no collectives.

Per-core algorithm (N=16384, C=10, D=16, 3 EM iterations, output = miu):
  - votes stored in SBUF f32 N-layout vv[p, n2, 0:160]=v, [160:320]=v^2,
    [320]=1.0, with n = p*128 + n2  (one sample at a time: 160.5KB/partition).
  - All n-reductions (M1=sum_n r*v, M2=sum_n r*v^2, R=sum_n r) are PE matmuls
    with the per-iteration routing weights r as the stationary operand
    (plain fp32 matmuls: exact, 4 cyc/row). The matmul
    produces the full 10x160 cross matrix; the needed diagonal c-blocks are
    extracted via a DRAM round-trip (diag AP [[176,10],[1,16]]).
  - sigma via moments: sigma = M2/R - miu^2*(2-S) + eps, S = R/(R+eps).
  - log-p d-contraction: q[n,c] = A2_c + sum_d v*g - sum_d v^2*s2 with
    g=miu/sigma, s2=1/(2*sigma) broadcast along partitions; computed by DVE
    tensor ops + segmented reduces.
  - The reference's per-n max-shift is replaced by the constant upper bound
    max_cd(h), h=-0.5*log(sigma) (difference enters only through eps, ~1e-21).
"""

import numpy as np

B, N, C, D = 16, 16384, 10, 16
CD = C * D
NB = 2          # samples per core
NCORES = 8
P = 128         # partitions
NCH = N // P    # 128 chunks; n = p*128 + n2
EPS = 1e-9
LN10 = float(np.log(10.0))
SLAB = 8        # chunks per z-slab
NSLAB = NCH // SLAB

_CACHE = {}


def _build():
    from contextlib import ExitStack
    import concourse.bass as bass
    import concourse.bacc as bacc
    import concourse.mybir as mybir
    import concourse.tile as tile
    import concourse.bass_isa as bass_isa

    f32 = mybir.dt.float32
    f32r = mybir.dt.float32r
    AX = mybir.AxisListType
    OP = mybir.AluOpType
    ACTF = mybir.ActivationFunctionType

    nc = bacc.Bacc("TRN2")
    votes_d = nc.dram_tensor("votes", [NB, N, C, D], f32, kind="ExternalInput")
    act_d = nc.dram_tensor("activation", [NB, N, 1], f32, kind="ExternalInput")
    out_d = nc.dram_tensor("miu_out", [NB, C, D], f32, kind="ExternalOutput")

    def bfree(t_ap, nrep):
        # read-broadcast an SBUF AP along a new middle free dim (step 0)
        return bass.AP(
            tensor=t_ap.tensor,
            offset=t_ap.offset,
            ap=[t_ap.ap[0], [0, nrep]] + list(t_ap.ap[1:]),
        )

    with ExitStack() as ctx:
        tc = ctx.enter_context(tile.TileContext(nc))
        big = ctx.enter_context(tc.tile_pool(name="big", bufs=1))
        zpool = ctx.enter_context(tc.tile_pool(name="z", bufs=3))
        qpool = ctx.enter_context(tc.tile_pool(name="q", bufs=1))
        rpool = ctx.enter_context(tc.tile_pool(name="r", bufs=1))
        small = ctx.enter_context(tc.tile_pool(name="small", bufs=1))
        tiny = ctx.enter_context(tc.tile_pool(name="tiny", bufs=1))
        psum = ctx.enter_context(tc.tile_pool(name="psum", bufs=2, space="PSUM"))

        # persistent across the whole kernel
        vv = big.tile([P, NCH, 321], f32)          # [v | v^2 | ones]
        a_t = big.tile([P, NCH], f32)              # activation
        act_b = big.tile([P, C], f32)              # act_out bcast (free c)

        for s in range(NB):
            # ---------------- load sample s ----------------
            if s > 0:
                nc.all_engine_barrier()
            vsrc = votes_d[s].rearrange("(p n2) c d -> p n2 (c d)", p=P)
            for g in range(16):
                sl = slice(g * SLAB, (g + 1) * SLAB)
                nc.gpsimd.dma_start(out=vv[:, sl, 0:160], in_=vsrc[:, sl, :])
                nc.scalar.activation(
                    out=vv[:, sl, 161:321], in_=vv[:, sl, 0:160], func=ACTF.Square
                )
                nc.scalar.activation(
                    out=vv[:, sl, 160:161], in_=vv[:, sl, 0:1],
                    func=ACTF.Copy, bias=1.0, scale=0.0,
                )
            nc.gpsimd.dma_start(
                out=a_t[:, :], in_=act_d[s].rearrange("(p n2) one -> p (n2 one)", p=P)
            )

            r_t = rpool.tile([P, NCH, C], f32, tag="r")
            q_t = qpool.tile([P, NCH, C], f32)

            miu_diag = None
            for it in range(3):
                # ---------------- r computation ----------------
                if it == 0:
                    t0 = small.tile([P, NCH], f32, tag="sm0")
                    nc.vector.tensor_scalar_add(t0, a_t, EPS)
                    t1 = small.tile([P, NCH], f32, tag="sm1")
                    nc.vector.reciprocal(t1, t0)
                    t2 = small.tile([P, NCH], f32, tag="sm2")
                    nc.vector.tensor_tensor(out=t2, in0=a_t, in1=t1, op=OP.mult)
                    # r0 = a/(C*(a+eps)) broadcast over c (on ACT so the
                    # first matmul's waits collapse to {ACT, DMA})
                    nc.scalar.activation(
                        r_t[:, :, :],
                        bass.AP(
                            tensor=t2.tensor,
                            offset=t2.offset,
                            ap=[t2.ap[0], t2.ap[1], [0, C]],
                        ),
                        func=ACTF.Copy,
                        scale=1.0 / C,
                    )
                    # canary: dummy matmul consuming an ACT-written tile so the
                    # real matmuls' PE-side waits collapse to one semaphore
                    cn = tiny.tile([1, 1], f32, tag="cn")
                    nc.scalar.activation(out=cn, in_=a_t[0:1, 0:1], func=ACTF.Copy)
                    psc = psum.tile([1, 1], f32, tag="psc")
                    nc.tensor.matmul(psc, cn, cn, start=True, stop=True)
                    mm = psum.tile([10, 321], f32, tag="mm")
                    for n2 in range(NCH):
                        nc.tensor.matmul(
                            mm,
                            r_t[:, n2, :],
                            vv[:, n2, :],
                            start=(n2 == 0),
                            stop=(n2 == NCH - 1),
                        )
                else:
                    # params from miu_diag/sig_diag (single-partition [1,160])
                    inv_sig = tiny.tile([1, CD], f32, tag="pinv")
                    nc.vector.reciprocal(inv_sig, sig_diag)
                    g_f = tiny.tile([1, CD], f32, tag="pg")
                    nc.vector.tensor_tensor(out=g_f, in0=miu_diag, in1=inv_sig, op=OP.mult)
                    s2_f = tiny.tile([1, CD], f32, tag="ps2")
                    nc.vector.tensor_scalar_mul(s2_f, inv_sig, 0.5)
                    ls_f = tiny.tile([1, CD], f32, tag="pls")
                    nc.scalar.activation(out=ls_f, in_=sig_diag, func=ACTF.Ln)
                    # u = log(sig) + miu^2/sig ; A = -0.5 * sum_d u per c
                    w_f = tiny.tile([1, CD], f32, tag="pw")
                    nc.vector.tensor_tensor(out=w_f, in0=miu_diag, in1=g_f, op=OP.mult)
                    u_f = tiny.tile([1, CD], f32, tag="pu")
                    nc.vector.tensor_tensor(out=u_f, in0=ls_f, in1=w_f, op=OP.add)
                    Ac = tiny.tile([1, C], f32, tag="pAc")
                    nc.vector.tensor_reduce(
                        Ac, u_f.rearrange("one (c d) -> one c d", c=C), axis=AX.X, op=OP.add
                    )
                    # m' = max_cd h = -0.5*min_cd log(sig)
                    lmin = tiny.tile([1, 1], f32, tag="plm")
                    nc.vector.tensor_reduce(lmin, ls_f, axis=AX.X, op=OP.min)
                    # A2_c = -0.5*Ac + D*(ln10 + 0.5*lmin)
                    bias_t = tiny.tile([1, 1], f32, tag="pbi")
                    nc.vector.tensor_scalar(
                        bias_t, lmin, 0.5 * float(D), float(D) * LN10, OP.mult, OP.add
                    )
                    A2 = tiny.tile([1, C], f32, tag="pA2")
                    nc.vector.tensor_scalar(A2, Ac, -0.5, bias_t, OP.mult, OP.add)
                    A2_128 = small.tile([P, C], f32, tag="a2b")
                    nc.gpsimd.partition_broadcast(A2_128, A2)
                    g_b = small.tile([P, CD], f32, tag="gb")
                    nc.gpsimd.partition_broadcast(g_b, g_f)
                    s2_b = small.tile([P, CD], f32, tag="s2b")
                    nc.gpsimd.partition_broadcast(s2_b, s2_f)

                    # ---------------- q volume + r chain + matmuls, blocked
                    # over n so PE matmuls of block k overlap the DVE chain
                    # of block k+1 ----------------------------------------
                    g_b3 = g_b.rearrange("p (c d) -> p c d", c=C)
                    s2_b3 = s2_b.rearrange("p (c d) -> p c d", c=C)
                    FW = 321 if it < 2 else 161
                    mm = psum.tile([10, 321], f32, tag="mm")
                    sc = small.tile([P, NCH], f32, tag="sm0")
                    d1 = small.tile([P, NCH], f32, tag="sm1")
                    d2 = small.tile([P, NCH], f32, tag="sm2")
                    fac1 = small.tile([P, NCH], f32, tag="sm3")
                    sc2 = small.tile([P, NCH], f32, tag="sm4")
                    d3 = small.tile([P, NCH], f32, tag="sm5")
                    d4 = small.tile([P, NCH], f32, tag="sm6")
                    NBLK = 4
                    BCH = NCH // NBLK
                    for blk in range(NBLK):
                        for sb in range(BCH // SLAB):
                            sl = slice(
                                blk * BCH + sb * SLAB, blk * BCH + (sb + 1) * SLAB
                            )
                            z1 = zpool.tile([P, SLAB, C, D], f32, tag="z")
                            nc.vector.tensor_tensor(
                                out=z1,
                                in0=vv[:, sl, 0:160].rearrange(
                                    "p s (c d) -> p s c d", c=C
                                ),
                                in1=bfree(g_b3, SLAB),
                                op=OP.mult,
                            )
                            z2 = zpool.tile([P, SLAB, C, D], f32, tag="z")
                            nc.gpsimd.tensor_tensor(
                                out=z2,
                                in0=vv[:, sl, 161:321].rearrange(
                                    "p s (c d) -> p s c d", c=C
                                ),
                                in1=bfree(s2_b3, SLAB),
                                op=OP.mult,
                            )
                            q1 = zpool.tile([P, SLAB, C], f32, tag="qs")
                            nc.vector.tensor_reduce(q1, z1, axis=AX.X, op=OP.add)
                            q2 = zpool.tile([P, SLAB, C], f32, tag="qs")
                            nc.vector.tensor_reduce(q2, z2, axis=AX.X, op=OP.add)
                            nc.vector.tensor_tensor(
                                out=q_t[:, sl, :], in0=q1, in1=q2, op=OP.subtract
                            )
                        bs = slice(blk * BCH, (blk + 1) * BCH)
                        qb = q_t[:, bs, :]
                        nc.vector.tensor_tensor(
                            out=qb, in0=qb, in1=bfree(A2_128, BCH), op=OP.add
                        )
                        nc.scalar.activation(out=qb, in_=qb, func=ACTF.Exp)
                        nc.vector.tensor_tensor(
                            out=qb, in0=qb, in1=bfree(act_b, BCH), op=OP.mult
                        )
                        nc.vector.tensor_reduce(sc[:, bs], qb, axis=AX.X, op=OP.add)
                        nc.vector.tensor_scalar_add(d1[:, bs], sc[:, bs], EPS)
                        nc.vector.reciprocal(d2[:, bs], d1[:, bs])
                        nc.vector.tensor_tensor(
                            out=fac1[:, bs], in0=d2[:, bs], in1=a_t[:, bs], op=OP.mult
                        )
                        f1s = fac1[:, bs]
                        nc.vector.tensor_tensor(
                            out=qb,
                            in0=qb,
                            in1=bass.AP(
                                tensor=f1s.tensor,
                                offset=f1s.offset,
                                ap=[f1s.ap[0], f1s.ap[1], [0, C]],
                            ),
                            op=OP.mult,
                        )
                        nc.vector.tensor_reduce(sc2[:, bs], qb, axis=AX.X, op=OP.add)
                        nc.vector.tensor_scalar_add(d3[:, bs], sc2[:, bs], EPS)
                        nc.vector.reciprocal(d4[:, bs], d3[:, bs])
                        d4s = d4[:, bs]
                        nc.vector.tensor_tensor(
                            out=r_t[:, bs, :],
                            in0=qb,
                            in1=bass.AP(
                                tensor=d4s.tensor,
                                offset=d4s.offset,
                                ap=[d4s.ap[0], d4s.ap[1], [0, C]],
                            ),
                            op=OP.mult,
                        )
                        for n2 in range(blk * BCH, (blk + 1) * BCH):
                            nc.tensor.matmul(
                                mm[:, 0:FW],
                                r_t[:, n2, :],
                                vv[:, n2, 0:FW],
                                start=(n2 == 0),
                                stop=(n2 == NCH - 1),
                            )


                # ---------------- stats (full 10x160 cross matrices) --------
                Rse = tiny.tile([10, 1], f32, tag="cRs")
                nc.vector.tensor_scalar_add(Rse, mm[:, 160:161], EPS)
                invR = tiny.tile([10, 1], f32, tag="cIR")
                nc.vector.reciprocal(invR, Rse)
                miu_full = small.tile([10, CD], f32, tag="miuf")
                nc.vector.tensor_scalar(miu_full, mm[:, 0:160], invR, None, OP.mult)

                if it < 2:
                    m2n = tiny.tile([10, CD], f32, tag="cm2")
                    nc.vector.tensor_scalar(m2n, mm[:, 161:321], invR, None, OP.mult)
                    S_t = tiny.tile([10, 1], f32, tag="cS")
                    nc.vector.tensor_scalar(S_t, mm[:, 160:161], invR, None, OP.mult)
                    c2_t = tiny.tile([10, 1], f32, tag="cc2")
                    nc.vector.tensor_scalar(c2_t, S_t, -1.0, 2.0, OP.mult, OP.add)
                    mmu = tiny.tile([10, CD], f32, tag="cmu")
                    nc.vector.tensor_tensor(out=mmu, in0=miu_full, in1=miu_full, op=OP.mult)
                    nc.vector.tensor_scalar(mmu, mmu, c2_t, None, OP.mult)
                    sig_full = tiny.tile([10, CD], f32, tag="csg")
                    nc.vector.tensor_scalar(sig_full, m2n, EPS, None, OP.add)
                    nc.vector.tensor_tensor(out=sig_full, in0=sig_full, in1=mmu, op=OP.subtract)

                    miu_diag = tiny.tile([1, CD], f32, tag="dgm")
                    sig_diag = tiny.tile([1, CD], f32, tag="dgs")
                    if it == 0:
                        # r0 is c-uniform: every row of the cross matrix is
                        # the diagonal -> plain same-partition copies
                        nc.vector.tensor_copy(miu_diag, miu_full[0:1, :])
                        nc.vector.tensor_copy(sig_diag, sig_full[0:1, :])
                    else:
                        # diag extraction via DRAM round-trip
                        scr = nc.dram_tensor(f"scr_{s}_{it}", [4096], f32, kind="Internal")[:]
                        nc.gpsimd.dma_start(out=scr[0:1600], in_=miu_full)
                        nc.gpsimd.dma_start(out=scr[1600:3200], in_=sig_full)
                        diag_miu_ap = bass.AP(
                            tensor=scr.tensor,
                            offset=scr.offset,
                            ap=[[0, 1], [176, 10], [1, 16]],
                        )
                        diag_sig_ap = bass.AP(
                            tensor=scr.tensor,
                            offset=scr.offset + 1600,
                            ap=[[0, 1], [176, 10], [1, 16]],
                        )
                        nc.gpsimd.dma_start(out=miu_diag, in_=diag_miu_ap)
                        nc.gpsimd.dma_start(out=sig_diag, in_=diag_sig_ap)

                    # act_out for next iter
                    if it == 0:
                        nc.vector.memset(act_b, 1.0 / C)
                    else:
                        nc.gpsimd.dma_start(out=scr[3200:3210], in_=Rse)
                        Rf = tiny.tile([1, C], f32, tag="pRf")
                        nc.gpsimd.dma_start(out=Rf, in_=scr[3200:3210])
                        mx = tiny.tile([1, 1], f32, tag="pmx")
                        nc.vector.tensor_reduce(mx, Rf, axis=AX.X, op=OP.max)
                        sh = tiny.tile([1, C], f32, tag="psh")
                        nc.vector.tensor_scalar(sh, Rf, mx, None, OP.subtract)
                        ex = tiny.tile([1, C], f32, tag="pex")
                        nc.scalar.activation(out=ex, in_=sh, func=ACTF.Exp)
                        sm = tiny.tile([1, 1], f32, tag="psm")
                        nc.vector.tensor_reduce(sm, ex, axis=AX.X, op=OP.add)
                        smr = tiny.tile([1, 1], f32, tag="psr")
                        nc.vector.reciprocal(smr, sm)
                        ao = tiny.tile([1, C], f32, tag="pao")
                        nc.vector.tensor_scalar(ao, ex, smr, None, OP.mult)
                        nc.gpsimd.partition_broadcast(act_b, ao)
                else:
                    # final: extract diag of miu_full -> output
                    scr = nc.dram_tensor(f"scr_{s}_{it}", [4096], f32, kind="Internal")[:]
                    nc.gpsimd.dma_start(out=scr[0:1600], in_=miu_full)
                    out_diag = bass.AP(
                        tensor=scr.tensor,
                        offset=scr.offset,
                        ap=[[0, 1], [176, 10], [1, 16]],
                    )
                    fin = tiny.tile([1, CD], f32, tag="dgm")
                    nc.gpsimd.dma_start(out=fin, in_=out_diag)
                    nc.gpsimd.dma_start(
                        out=out_d[s].rearrange("c d -> (c d)"), in_=fin
                    )

    nc.compile()
    return nc


def kernel(votes, activation, beta_v, beta_a):
    from concourse.bass_utils import run_bass_kernel_spmd

    if "nc" not in _CACHE:
        _CACHE["nc"] = _build()
    nc = _CACHE["nc"]

    votes = np.ascontiguousarray(votes, dtype=np.float32)
    activation = np.ascontiguousarray(activation, dtype=np.float32)
    in_maps = [
        {
            "votes": votes[i * NB : (i + 1) * NB],
            "activation": activation[i * NB : (i + 1) * NB],
        }
        for i in range(NCORES)
    ]
    res = run_bass_kernel_spmd(nc, in_maps, core_ids=list(range(NCORES)))
    out = np.concatenate([res.results[i]["miu_out"] for i in range(NCORES)], axis=0)
    return out.reshape(B, 1, C, D).astype(np.float32)


if __name__ == "__main__":
    _build()
    print("build OK")


# revision 23
# speedup vs baseline: 1.2567x; 1.1081x over previous
"""EM-routing (DigitCaps) kernel for 8 trn2 NeuronCores.

Sharding: data-parallel over batch B=16 -> 2 samples per core, no collectives.

Per-core algorithm (N=16384, C=10, D=16, 3 EM iterations, output = miu):
  - votes stored in SBUF f32 N-layout vv[p, n2, 0:160]=v, [160:320]=v^2,
    [320]=1.0, with n = p*128 + n2  (one sample at a time: 160.5KB/partition).
  - All n-reductions (M1=sum_n r*v, M2=sum_n r*v^2, R=sum_n r) are PE matmuls
    with the per-iteration routing weights r as the stationary operand
    (plain fp32 matmuls: exact, 4 cyc/row). The matmul
    produces the full 10x160 cross matrix; the needed diagonal c-blocks are
    extracted via a DRAM round-trip (diag AP [[176,10],[1,16]]).
  - sigma via moments: sigma = M2/R - miu^2*(2-S) + eps, S = R/(R+eps).
  - log-p d-contraction: q[n,c] = A2_c + sum_d v*g - sum_d v^2*s2 with
    g=miu/sigma, s2=1/(2*sigma) broadcast along partitions; computed by DVE
    tensor ops + segmented reduces.
  - The reference's per-n max-shift is replaced by the constant upper bound
    max_cd(h), h=-0.5*log(sigma) (difference enters only through eps, ~1e-21).
"""

import numpy as np

B, N, C, D = 16, 16384, 10, 16
CD = C * D
NB = 2          # samples per core
NCORES = 8
P = 128         # partitions
NCH = N // P    # 128 chunks; n = p*128 + n2
EPS = 1e-9
LN10 = float(np.log(10.0))
SLAB = 8        # chunks per z-slab
NSLAB = NCH // SLAB

_CACHE = {}


def _build():
    from contextlib import ExitStack
    import concourse.bass as bass
    import concourse.bacc as bacc
    import concourse.mybir as mybir
    import concourse.tile as tile
    import concourse.bass_isa as bass_isa

    f32 = mybir.dt.float32
    f32r = mybir.dt.float32r
    AX = mybir.AxisListType
    OP = mybir.AluOpType
    ACTF = mybir.ActivationFunctionType

    nc = bacc.Bacc("TRN2")
    votes_d = nc.dram_tensor("votes", [NB, N, C, D], f32, kind="ExternalInput")
    act_d = nc.dram_tensor("activation", [NB, N, 1], f32, kind="ExternalInput")
    out_d = nc.dram_tensor("miu_out", [NB, C, D], f32, kind="ExternalOutput")

    def bfree(t_ap, nrep):
        # read-broadcast an SBUF AP along a new middle free dim (step 0)
        return bass.AP(
            tensor=t_ap.tensor,
            offset=t_ap.offset,
            ap=[t_ap.ap[0], [0, nrep]] + list(t_ap.ap[1:]),
        )

    with ExitStack() as ctx:
        tc = ctx.enter_context(tile.TileContext(nc))
        big = ctx.enter_context(tc.tile_pool(name="big", bufs=1))
        zpool = ctx.enter_context(tc.tile_pool(name="z", bufs=3))
        qpool = ctx.enter_context(tc.tile_pool(name="q", bufs=1))
        rpool = ctx.enter_context(tc.tile_pool(name="r", bufs=1))
        small = ctx.enter_context(tc.tile_pool(name="small", bufs=1))
        tiny = ctx.enter_context(tc.tile_pool(name="tiny", bufs=1))
        psum = ctx.enter_context(tc.tile_pool(name="psum", bufs=2, space="PSUM"))

        # persistent across the whole kernel
        vv = big.tile([P, NCH, 321], f32)          # [v | v^2 | ones]
        a_t = big.tile([P, NCH], f32)              # activation
        act_b = big.tile([P, C], f32)              # act_out bcast (free c)

        for s in range(NB):
            # ---------------- load sample s ----------------
            if s > 0:
                nc.all_engine_barrier()
            nc.gpsimd.dma_start(
                out=a_t[:, :], in_=act_d[s].rearrange("(p n2) one -> p (n2 one)", p=P)
            )
            vsrc = votes_d[s].rearrange("(p n2) c d -> p n2 (c d)", p=P)
            for g in range(16):
                sl = slice(g * SLAB, (g + 1) * SLAB)
                nc.gpsimd.dma_start(out=vv[:, sl, 0:160], in_=vsrc[:, sl, :])
                nc.scalar.activation(
                    out=vv[:, sl, 161:321], in_=vv[:, sl, 0:160], func=ACTF.Square
                )
                nc.scalar.activation(
                    out=vv[:, sl, 160:161], in_=vv[:, sl, 0:1],
                    func=ACTF.Copy, bias=1.0, scale=0.0,
                )

            r_t = rpool.tile([P, NCH, C], f32, tag="r")
            q_t = qpool.tile([P, NCH, C], f32)

            miu_diag = None
            for it in range(3):
                # ---------------- r computation ----------------
                if it == 0:
                    t0 = small.tile([P, NCH], f32, tag="sm0")
                    nc.vector.tensor_scalar_add(t0, a_t, EPS)
                    t1 = small.tile([P, NCH], f32, tag="sm1")
                    nc.vector.reciprocal(t1, t0)
                    t2 = small.tile([P, NCH], f32, tag="sm2")
                    nc.vector.tensor_tensor(out=t2, in0=a_t, in1=t1, op=OP.mult)
                    # r0 = a/(C*(a+eps)) broadcast over c (on ACT so the
                    # first matmul's waits collapse to {ACT, DMA})
                    nc.scalar.activation(
                        r_t[:, :, :],
                        bass.AP(
                            tensor=t2.tensor,
                            offset=t2.offset,
                            ap=[t2.ap[0], t2.ap[1], [0, C]],
                        ),
                        func=ACTF.Copy,
                        scale=1.0 / C,
                    )
                    # canary: dummy matmul consuming an ACT-written tile so the
                    # real matmuls' PE-side waits collapse to one semaphore
                    cn = tiny.tile([1, 1], f32, tag="cn")
                    nc.scalar.activation(out=cn, in_=a_t[0:1, 0:1], func=ACTF.Copy)
                    psc = psum.tile([1, 1], f32, tag="psc")
                    nc.tensor.matmul(psc, cn, cn, start=True, stop=True)
                    mm = psum.tile([10, 321], f32, tag="mm")
                    for n2 in range(NCH):
                        nc.tensor.matmul(
                            mm,
                            r_t[:, n2, :],
                            vv[:, n2, :],
                            start=(n2 == 0),
                            stop=(n2 == NCH - 1),
                        )
                else:
                    # params from miu_diag/sig_diag (single-partition [1,160])
                    inv_sig = tiny.tile([1, CD], f32, tag="pinv")
                    nc.vector.reciprocal(inv_sig, sig_diag)
                    g_f = tiny.tile([1, CD], f32, tag="pg")
                    nc.vector.tensor_tensor(out=g_f, in0=miu_diag, in1=inv_sig, op=OP.mult)
                    s2_f = tiny.tile([1, CD], f32, tag="ps2")
                    nc.vector.tensor_scalar_mul(s2_f, inv_sig, 0.5)
                    ls_f = tiny.tile([1, CD], f32, tag="pls")
                    nc.scalar.activation(out=ls_f, in_=sig_diag, func=ACTF.Ln)
                    # u = log(sig) + miu^2/sig ; A = -0.5 * sum_d u per c
                    w_f = tiny.tile([1, CD], f32, tag="pw")
                    nc.vector.tensor_tensor(out=w_f, in0=miu_diag, in1=g_f, op=OP.mult)
                    u_f = tiny.tile([1, CD], f32, tag="pu")
                    nc.vector.tensor_tensor(out=u_f, in0=ls_f, in1=w_f, op=OP.add)
                    Ac = tiny.tile([1, C], f32, tag="pAc")
                    nc.vector.tensor_reduce(
                        Ac, u_f.rearrange("one (c d) -> one c d", c=C), axis=AX.X, op=OP.add
                    )
                    # m' = max_cd h = -0.5*min_cd log(sig)
                    lmin = tiny.tile([1, 1], f32, tag="plm")
                    nc.vector.tensor_reduce(lmin, ls_f, axis=AX.X, op=OP.min)
                    # A2_c = -0.5*Ac + D*(ln10 + 0.5*lmin)
                    bias_t = tiny.tile([1, 1], f32, tag="pbi")
                    nc.vector.tensor_scalar(
                        bias_t, lmin, 0.5 * float(D), float(D) * LN10, OP.mult, OP.add
                    )
                    A2 = tiny.tile([1, C], f32, tag="pA2")
                    nc.vector.tensor_scalar(A2, Ac, -0.5, bias_t, OP.mult, OP.add)
                    A2_128 = small.tile([P, C], f32, tag="a2b")
                    nc.gpsimd.partition_broadcast(A2_128, A2)
                    g_b = small.tile([P, CD], f32, tag="gb")
                    nc.gpsimd.partition_broadcast(g_b, g_f)
                    s2_b = small.tile([P, CD], f32, tag="s2b")
                    nc.gpsimd.partition_broadcast(s2_b, s2_f)

                    # ---------------- q volume + r chain + matmuls, blocked
                    # over n so PE matmuls of block k overlap the DVE chain
                    # of block k+1 ----------------------------------------
                    g_b3 = g_b.rearrange("p (c d) -> p c d", c=C)
                    s2_b3 = s2_b.rearrange("p (c d) -> p c d", c=C)
                    FW = 321 if it < 2 else 161
                    mm = psum.tile([10, 321], f32, tag="mm")
                    sc = small.tile([P, NCH], f32, tag="sm0")
                    d1 = small.tile([P, NCH], f32, tag="sm1")
                    d2 = small.tile([P, NCH], f32, tag="sm2")
                    fac1 = small.tile([P, NCH], f32, tag="sm3")
                    sc2 = small.tile([P, NCH], f32, tag="sm4")
                    d3 = small.tile([P, NCH], f32, tag="sm5")
                    d4 = small.tile([P, NCH], f32, tag="sm6")
                    NBLK = 4
                    BCH = NCH // NBLK
                    for blk in range(NBLK):
                        for sb in range(BCH // SLAB):
                            sl = slice(
                                blk * BCH + sb * SLAB, blk * BCH + (sb + 1) * SLAB
                            )
                            z1 = zpool.tile([P, SLAB, C, D], f32, tag="z")
                            nc.vector.tensor_tensor(
                                out=z1,
                                in0=vv[:, sl, 0:160].rearrange(
                                    "p s (c d) -> p s c d", c=C
                                ),
                                in1=bfree(g_b3, SLAB),
                                op=OP.mult,
                            )
                            z2 = zpool.tile([P, SLAB, C, D], f32, tag="z")
                            nc.gpsimd.tensor_tensor(
                                out=z2,
                                in0=vv[:, sl, 161:321].rearrange(
                                    "p s (c d) -> p s c d", c=C
                                ),
                                in1=bfree(s2_b3, SLAB),
                                op=OP.mult,
                            )
                            q1 = zpool.tile([P, SLAB, C], f32, tag="qs")
                            nc.vector.tensor_reduce(q1, z1, axis=AX.X, op=OP.add)
                            q2 = zpool.tile([P, SLAB, C], f32, tag="qs")
                            nc.vector.tensor_reduce(q2, z2, axis=AX.X, op=OP.add)
                            nc.gpsimd.tensor_tensor(
                                out=q_t[:, sl, :], in0=q1, in1=q2, op=OP.subtract
                            )
                        bs = slice(blk * BCH, (blk + 1) * BCH)
                        qb = q_t[:, bs, :]
                        nc.vector.tensor_tensor(
                            out=qb, in0=qb, in1=bfree(A2_128, BCH), op=OP.add
                        )
                        nc.scalar.activation(out=qb, in_=qb, func=ACTF.Exp)
                        nc.vector.tensor_tensor(
                            out=qb, in0=qb, in1=bfree(act_b, BCH), op=OP.mult
                        )
                        nc.vector.tensor_reduce(sc[:, bs], qb, axis=AX.X, op=OP.add)
                        nc.vector.tensor_scalar_add(d1[:, bs], sc[:, bs], EPS)
                        nc.vector.reciprocal(d2[:, bs], d1[:, bs])
                        nc.vector.tensor_tensor(
                            out=fac1[:, bs], in0=d2[:, bs], in1=a_t[:, bs], op=OP.mult
                        )
                        f1s = fac1[:, bs]
                        nc.vector.tensor_tensor(
                            out=qb,
                            in0=qb,
                            in1=bass.AP(
                                tensor=f1s.tensor,
                                offset=f1s.offset,
                                ap=[f1s.ap[0], f1s.ap[1], [0, C]],
                            ),
                            op=OP.mult,
                        )
                        nc.vector.tensor_reduce(sc2[:, bs], qb, axis=AX.X, op=OP.add)
                        nc.vector.tensor_scalar_add(d3[:, bs], sc2[:, bs], EPS)
                        nc.vector.reciprocal(d4[:, bs], d3[:, bs])
                        d4s = d4[:, bs]
                        nc.vector.tensor_tensor(
                            out=r_t[:, bs, :],
                            in0=qb,
                            in1=bass.AP(
                                tensor=d4s.tensor,
                                offset=d4s.offset,
                                ap=[d4s.ap[0], d4s.ap[1], [0, C]],
                            ),
                            op=OP.mult,
                        )
                        for n2 in range(blk * BCH, (blk + 1) * BCH):
                            nc.tensor.matmul(
                                mm[:, 0:FW],
                                r_t[:, n2, :],
                                vv[:, n2, 0:FW],
                                start=(n2 == 0),
                                stop=(n2 == NCH - 1),
                            )


                # ---------------- stats (full 10x160 cross matrices) --------
                Rse = tiny.tile([10, 1], f32, tag="cRs")
                nc.vector.tensor_scalar_add(Rse, mm[:, 160:161], EPS)
                invR = tiny.tile([10, 1], f32, tag="cIR")
                nc.vector.reciprocal(invR, Rse)
                miu_full = small.tile([10, CD], f32, tag="miuf")
                nc.vector.tensor_scalar(miu_full, mm[:, 0:160], invR, None, OP.mult)

                if it < 2:
                    m2n = tiny.tile([10, CD], f32, tag="cm2")
                    nc.vector.tensor_scalar(m2n, mm[:, 161:321], invR, None, OP.mult)
                    S_t = tiny.tile([10, 1], f32, tag="cS")
                    nc.vector.tensor_scalar(S_t, mm[:, 160:161], invR, None, OP.mult)
                    c2_t = tiny.tile([10, 1], f32, tag="cc2")
                    nc.vector.tensor_scalar(c2_t, S_t, -1.0, 2.0, OP.mult, OP.add)
                    mmu = tiny.tile([10, CD], f32, tag="cmu")
                    nc.vector.tensor_tensor(out=mmu, in0=miu_full, in1=miu_full, op=OP.mult)
                    nc.vector.tensor_scalar(mmu, mmu, c2_t, None, OP.mult)
                    sig_full = tiny.tile([10, CD], f32, tag="csg")
                    nc.vector.tensor_scalar(sig_full, m2n, EPS, None, OP.add)
                    nc.vector.tensor_tensor(out=sig_full, in0=sig_full, in1=mmu, op=OP.subtract)

                    miu_diag = tiny.tile([1, CD], f32, tag="dgm")
                    sig_diag = tiny.tile([1, CD], f32, tag="dgs")
                    if it == 0:
                        # r0 is c-uniform: every row of the cross matrix is
                        # the diagonal -> plain same-partition copies
                        nc.vector.tensor_copy(miu_diag, miu_full[0:1, :])
                        nc.vector.tensor_copy(sig_diag, sig_full[0:1, :])
                    else:
                        # diag extraction via DRAM round-trip
                        scr = nc.dram_tensor(f"scr_{s}_{it}", [4096], f32, kind="Internal")[:]
                        nc.gpsimd.dma_start(out=scr[0:1600], in_=miu_full)
                        nc.gpsimd.dma_start(out=scr[1600:3200], in_=sig_full)
                        diag_miu_ap = bass.AP(
                            tensor=scr.tensor,
                            offset=scr.offset,
                            ap=[[0, 1], [176, 10], [1, 16]],
                        )
                        diag_sig_ap = bass.AP(
                            tensor=scr.tensor,
                            offset=scr.offset + 1600,
                            ap=[[0, 1], [176, 10], [1, 16]],
                        )
                        nc.gpsimd.dma_start(out=miu_diag, in_=diag_miu_ap)
                        nc.gpsimd.dma_start(out=sig_diag, in_=diag_sig_ap)

                    # act_out for next iter
                    if it == 0:
                        nc.vector.memset(act_b, 1.0 / C)
                    else:
                        nc.gpsimd.dma_start(out=scr[3200:3210], in_=Rse)
                        Rf = tiny.tile([1, C], f32, tag="pRf")
                        nc.gpsimd.dma_start(out=Rf, in_=scr[3200:3210])
                        mx = tiny.tile([1, 1], f32, tag="pmx")
                        nc.vector.tensor_reduce(mx, Rf, axis=AX.X, op=OP.max)
                        sh = tiny.tile([1, C], f32, tag="psh")
                        nc.vector.tensor_scalar(sh, Rf, mx, None, OP.subtract)
                        ex = tiny.tile([1, C], f32, tag="pex")
                        nc.scalar.activation(out=ex, in_=sh, func=ACTF.Exp)
                        sm = tiny.tile([1, 1], f32, tag="psm")
                        nc.vector.tensor_reduce(sm, ex, axis=AX.X, op=OP.add)
                        smr = tiny.tile([1, 1], f32, tag="psr")
                        nc.vector.reciprocal(smr, sm)
                        ao = tiny.tile([1, C], f32, tag="pao")
                        nc.vector.tensor_scalar(ao, ex, smr, None, OP.mult)
                        nc.gpsimd.partition_broadcast(act_b, ao)
                else:
                    # final: extract diag of miu_full -> output
                    scr = nc.dram_tensor(f"scr_{s}_{it}", [4096], f32, kind="Internal")[:]
                    nc.gpsimd.dma_start(out=scr[0:1600], in_=miu_full)
                    out_diag = bass.AP(
                        tensor=scr.tensor,
                        offset=scr.offset,
                        ap=[[0, 1], [176, 10], [1, 16]],
                    )
                    fin = tiny.tile([1, CD], f32, tag="dgm")
                    nc.gpsimd.dma_start(out=fin, in_=out_diag)
                    nc.gpsimd.dma_start(
                        out=out_d[s].rearrange("c d -> (c d)"), in_=fin
                    )

    nc.compile()
    return nc


def kernel(votes, activation, beta_v, beta_a):
    from concourse.bass_utils import run_bass_kernel_spmd

    if "nc" not in _CACHE:
        _CACHE["nc"] = _build()
    nc = _CACHE["nc"]

    votes = np.ascontiguousarray(votes, dtype=np.float32)
    activation = np.ascontiguousarray(activation, dtype=np.float32)
    in_maps = [
        {
            "votes": votes[i * NB : (i + 1) * NB],
            "activation": activation[i * NB : (i + 1) * NB],
        }
        for i in range(NCORES)
    ]
    res = run_bass_kernel_spmd(nc, in_maps, core_ids=list(range(NCORES)))
    out = np.concatenate([res.results[i]["miu_out"] for i in range(NCORES)], axis=0)
    return out.reshape(B, 1, C, D).astype(np.float32)


if __name__ == "__main__":
    _build()
    print("build OK")


# revision 27
# speedup vs baseline: 1.4356x; 1.1424x over previous
"""EM-routing (DigitCaps) kernel for 8 trn2 NeuronCores.

Sharding: data-parallel over batch B=16 -> 2 samples per core, no collectives.

Per-core algorithm (N=16384, C=10, D=16, 3 EM iterations, output = miu):
  - votes stored in SBUF f32 N-layout vv[p, n2, 0:160]=v, [160:320]=v^2,
    [320]=1.0, with n = p*128 + n2  (one sample at a time: 160.5KB/partition).
  - All n-reductions (M1=sum_n r*v, M2=sum_n r*v^2, R=sum_n r) are PE matmuls
    with the per-iteration routing weights r as the stationary operand
    (plain fp32 matmuls: exact, 4 cyc/row). The matmul
    produces the full 10x160 cross matrix; the needed diagonal c-blocks are
    extracted via a DRAM round-trip (diag AP [[176,10],[1,16]]).
  - sigma via moments: sigma = M2/R - miu^2*(2-S) + eps, S = R/(R+eps).
  - log-p d-contraction: q[n,c] = A2_c + sum_d v*g - sum_d v^2*s2 with
    g=miu/sigma, s2=1/(2*sigma) broadcast along partitions; computed by DVE
    tensor ops + segmented reduces.
  - The reference's per-n max-shift is replaced by the constant upper bound
    max_cd(h), h=-0.5*log(sigma) (difference enters only through eps, ~1e-21).
"""

import numpy as np

B, N, C, D = 16, 16384, 10, 16
CD = C * D
NB = 2          # samples per core
NCORES = 8
P = 128         # partitions
NCH = N // P    # 128 chunks; n = p*128 + n2
EPS = 1e-9
LN10 = float(np.log(10.0))
SLAB = 8        # chunks per z-slab
NSLAB = NCH // SLAB

_CACHE = {}


def _build():
    from contextlib import ExitStack
    import concourse.bass as bass
    import concourse.bacc as bacc
    import concourse.mybir as mybir
    import concourse.tile as tile
    import concourse.bass_isa as bass_isa

    f32 = mybir.dt.float32
    f32r = mybir.dt.float32r
    AX = mybir.AxisListType
    OP = mybir.AluOpType
    ACTF = mybir.ActivationFunctionType

    nc = bacc.Bacc("TRN2")
    votes_d = nc.dram_tensor("votes", [NB, N, C, D], f32, kind="ExternalInput")
    act_d = nc.dram_tensor("activation", [NB, N, 1], f32, kind="ExternalInput")
    out_d = nc.dram_tensor("miu_out", [NB, C, D], f32, kind="ExternalOutput")

    def bfree(t_ap, nrep):
        # read-broadcast an SBUF AP along a new middle free dim (step 0)
        return bass.AP(
            tensor=t_ap.tensor,
            offset=t_ap.offset,
            ap=[t_ap.ap[0], [0, nrep]] + list(t_ap.ap[1:]),
        )

    with ExitStack() as ctx:
        tc = ctx.enter_context(tile.TileContext(nc))
        big = ctx.enter_context(tc.tile_pool(name="big", bufs=1))
        zpool = ctx.enter_context(tc.tile_pool(name="z", bufs=3))
        qpool = ctx.enter_context(tc.tile_pool(name="q", bufs=1))
        rpool = ctx.enter_context(tc.tile_pool(name="r", bufs=1))
        small = ctx.enter_context(tc.tile_pool(name="small", bufs=1))
        tiny = ctx.enter_context(tc.tile_pool(name="tiny", bufs=1))
        psum = ctx.enter_context(tc.tile_pool(name="psum", bufs=2, space="PSUM"))

        # persistent across the whole kernel
        vv = big.tile([P, NCH, 321], f32)          # [v | v^2 | ones]
        a_t = big.tile([P, NCH], f32)              # activation
        act_b = big.tile([P, C], f32)              # act_out bcast (free c)

        for s in range(NB):
            # ---------------- load sample s ----------------
            if s > 0:
                nc.all_engine_barrier()
            nc.gpsimd.dma_start(
                out=a_t[:, :], in_=act_d[s].rearrange("(p n2) one -> p (n2 one)", p=P)
            )
            vsrc = votes_d[s].rearrange("(p n2) c d -> p n2 (c d)", p=P)
            for g in range(16):
                sl = slice(g * SLAB, (g + 1) * SLAB)
                nc.gpsimd.dma_start(out=vv[:, sl, 0:160], in_=vsrc[:, sl, :])
                nc.scalar.activation(
                    out=vv[:, sl, 161:321], in_=vv[:, sl, 0:160], func=ACTF.Square
                )
                nc.scalar.activation(
                    out=vv[:, sl, 160:161], in_=vv[:, sl, 0:1],
                    func=ACTF.Copy, bias=1.0, scale=0.0,
                )

            r_t = rpool.tile([P, NCH, C], f32, tag="r")
            q_t = qpool.tile([P, NCH, C], f32)

            miu_diag = None
            for it in range(3):
                # ---------------- r computation ----------------
                if it == 0:
                    t0 = small.tile([P, NCH], f32, tag="sm0")
                    nc.vector.tensor_scalar_add(t0, a_t, EPS)
                    t1 = small.tile([P, NCH], f32, tag="sm1")
                    nc.vector.reciprocal(t1, t0)
                    t2 = small.tile([P, NCH], f32, tag="sm2")
                    nc.vector.tensor_tensor(out=t2, in0=a_t, in1=t1, op=OP.mult)
                    # r0 = a/(C*(a+eps)) broadcast over c (on ACT so the
                    # first matmul's waits collapse to {ACT, DMA})
                    nc.scalar.activation(
                        r_t[:, :, :],
                        bass.AP(
                            tensor=t2.tensor,
                            offset=t2.offset,
                            ap=[t2.ap[0], t2.ap[1], [0, C]],
                        ),
                        func=ACTF.Copy,
                        scale=1.0 / C,
                    )
                    # canary: dummy matmul consuming an ACT-written tile so the
                    # real matmuls' PE-side waits collapse to one semaphore
                    cn = tiny.tile([1, 1], f32, tag="cn")
                    nc.scalar.activation(out=cn, in_=a_t[0:1, 0:1], func=ACTF.Copy)
                    psc = psum.tile([1, 1], f32, tag="psc")
                    nc.tensor.matmul(psc, cn, cn, start=True, stop=True)
                    mm = psum.tile([10, 321], f32, tag="mm")
                    for n2 in range(NCH):
                        nc.tensor.matmul(
                            mm,
                            r_t[:, n2, :],
                            vv[:, n2, :],
                            start=(n2 == 0),
                            stop=(n2 == NCH - 1),
                        )
                else:
                    # params from miu_diag/sig_diag (single-partition [1,160])
                    inv_sig = tiny.tile([1, CD], f32, tag="pinv")
                    nc.vector.reciprocal(inv_sig, sig_diag)
                    g_f = tiny.tile([1, CD], f32, tag="pg")
                    nc.vector.tensor_tensor(out=g_f, in0=miu_diag, in1=inv_sig, op=OP.mult)
                    s2_f = tiny.tile([1, CD], f32, tag="ps2")
                    nc.vector.tensor_scalar_mul(s2_f, inv_sig, 0.5)
                    ls_f = tiny.tile([1, CD], f32, tag="pls")
                    nc.scalar.activation(out=ls_f, in_=sig_diag, func=ACTF.Ln)
                    # u = log(sig) + miu^2/sig ; A = -0.5 * sum_d u per c
                    w_f = tiny.tile([1, CD], f32, tag="pw")
                    nc.vector.tensor_tensor(out=w_f, in0=miu_diag, in1=g_f, op=OP.mult)
                    u_f = tiny.tile([1, CD], f32, tag="pu")
                    nc.vector.tensor_tensor(out=u_f, in0=ls_f, in1=w_f, op=OP.add)
                    Ac = tiny.tile([1, C], f32, tag="pAc")
                    nc.vector.tensor_reduce(
                        Ac, u_f.rearrange("one (c d) -> one c d", c=C), axis=AX.X, op=OP.add
                    )
                    # m' = max_cd h = -0.5*min_cd log(sig)
                    lmin = tiny.tile([1, 1], f32, tag="plm")
                    nc.vector.tensor_reduce(lmin, ls_f, axis=AX.X, op=OP.min)
                    # A2_c = -0.5*Ac + D*(ln10 + 0.5*lmin)
                    bias_t = tiny.tile([1, 1], f32, tag="pbi")
                    nc.vector.tensor_scalar(
                        bias_t, lmin, 0.5 * float(D), float(D) * LN10, OP.mult, OP.add
                    )
                    A2 = tiny.tile([1, C], f32, tag="pA2")
                    nc.vector.tensor_scalar(A2, Ac, -0.5, bias_t, OP.mult, OP.add)
                    A2_128 = small.tile([P, C], f32, tag="a2b")
                    nc.gpsimd.partition_broadcast(A2_128, A2)
                    g_b = small.tile([P, CD], f32, tag="gb")
                    nc.gpsimd.partition_broadcast(g_b, g_f)
                    s2_b = small.tile([P, CD], f32, tag="s2b")
                    nc.gpsimd.partition_broadcast(s2_b, s2_f)

                    # ---------------- q volume + r chain + matmuls, blocked
                    # over n so PE matmuls of block k overlap the DVE chain
                    # of block k+1 ----------------------------------------
                    g_b3 = g_b.rearrange("p (c d) -> p c d", c=C)
                    s2_b3 = s2_b.rearrange("p (c d) -> p c d", c=C)
                    FW = 321 if it < 2 else 161
                    mm = psum.tile([10, 321], f32, tag="mm")
                    sc = small.tile([P, NCH], f32, tag="sm0")
                    d1 = small.tile([P, NCH], f32, tag="sm1")
                    d2 = small.tile([P, NCH], f32, tag="sm2")
                    fac1 = small.tile([P, NCH], f32, tag="sm3")
                    sc2 = small.tile([P, NCH], f32, tag="sm4")
                    d3 = small.tile([P, NCH], f32, tag="sm5")
                    d4 = small.tile([P, NCH], f32, tag="sm6")
                    NBLK = 16
                    BCH = NCH // NBLK
                    for blk in range(NBLK):
                        for sb in range(BCH // SLAB):
                            sl = slice(
                                blk * BCH + sb * SLAB, blk * BCH + (sb + 1) * SLAB
                            )
                            z1 = zpool.tile([P, SLAB, C, D], f32, tag="z")
                            nc.vector.tensor_tensor(
                                out=z1,
                                in0=vv[:, sl, 0:160].rearrange(
                                    "p s (c d) -> p s c d", c=C
                                ),
                                in1=bfree(g_b3, SLAB),
                                op=OP.mult,
                            )
                            z2 = zpool.tile([P, SLAB, C, D], f32, tag="z")
                            nc.gpsimd.tensor_tensor(
                                out=z2,
                                in0=vv[:, sl, 161:321].rearrange(
                                    "p s (c d) -> p s c d", c=C
                                ),
                                in1=bfree(s2_b3, SLAB),
                                op=OP.mult,
                            )
                            q1 = zpool.tile([P, SLAB, C], f32, tag="qs")
                            nc.vector.tensor_reduce(q1, z1, axis=AX.X, op=OP.add)
                            q2 = zpool.tile([P, SLAB, C], f32, tag="qs")
                            nc.vector.tensor_reduce(q2, z2, axis=AX.X, op=OP.add)
                            nc.gpsimd.tensor_tensor(
                                out=q_t[:, sl, :], in0=q1, in1=q2, op=OP.subtract
                            )
                        bs = slice(blk * BCH, (blk + 1) * BCH)
                        qb = q_t[:, bs, :]
                        nc.vector.tensor_tensor(
                            out=qb, in0=qb, in1=bfree(A2_128, BCH), op=OP.add
                        )
                        nc.scalar.activation(out=qb, in_=qb, func=ACTF.Exp)
                        nc.vector.tensor_tensor(
                            out=qb, in0=qb, in1=bfree(act_b, BCH), op=OP.mult
                        )
                        nc.vector.tensor_reduce(sc[:, bs], qb, axis=AX.X, op=OP.add)
                        nc.vector.tensor_scalar_add(d1[:, bs], sc[:, bs], EPS)
                        nc.vector.reciprocal(d2[:, bs], d1[:, bs])
                        nc.vector.tensor_tensor(
                            out=fac1[:, bs], in0=d2[:, bs], in1=a_t[:, bs], op=OP.mult
                        )
                        f1s = fac1[:, bs]
                        nc.vector.tensor_tensor(
                            out=qb,
                            in0=qb,
                            in1=bass.AP(
                                tensor=f1s.tensor,
                                offset=f1s.offset,
                                ap=[f1s.ap[0], f1s.ap[1], [0, C]],
                            ),
                            op=OP.mult,
                        )
                        nc.vector.tensor_reduce(sc2[:, bs], qb, axis=AX.X, op=OP.add)
                        nc.vector.tensor_scalar_add(d3[:, bs], sc2[:, bs], EPS)
                        nc.vector.reciprocal(d4[:, bs], d3[:, bs])
                        d4s = d4[:, bs]
                        nc.vector.tensor_tensor(
                            out=r_t[:, bs, :],
                            in0=qb,
                            in1=bass.AP(
                                tensor=d4s.tensor,
                                offset=d4s.offset,
                                ap=[d4s.ap[0], d4s.ap[1], [0, C]],
                            ),
                            op=OP.mult,
                        )
                        for n2 in range(blk * BCH, (blk + 1) * BCH):
                            nc.tensor.matmul(
                                mm[:, 0:FW],
                                r_t[:, n2, :],
                                vv[:, n2, 0:FW],
                                start=(n2 == 0),
                                stop=(n2 == NCH - 1),
                            )


                # ---------------- stats (full 10x160 cross matrices) --------
                Rse = tiny.tile([10, 1], f32, tag="cRs")
                nc.vector.tensor_scalar_add(Rse, mm[:, 160:161], EPS)
                invR = tiny.tile([10, 1], f32, tag="cIR")
                nc.vector.reciprocal(invR, Rse)
                miu_full = small.tile([10, CD], f32, tag="miuf")
                nc.vector.tensor_scalar(miu_full, mm[:, 0:160], invR, None, OP.mult)

                if it < 2:
                    m2n = tiny.tile([10, CD], f32, tag="cm2")
                    nc.vector.tensor_scalar(m2n, mm[:, 161:321], invR, None, OP.mult)
                    S_t = tiny.tile([10, 1], f32, tag="cS")
                    nc.vector.tensor_scalar(S_t, mm[:, 160:161], invR, None, OP.mult)
                    c2_t = tiny.tile([10, 1], f32, tag="cc2")
                    nc.vector.tensor_scalar(c2_t, S_t, -1.0, 2.0, OP.mult, OP.add)
                    mmu = tiny.tile([10, CD], f32, tag="cmu")
                    nc.vector.tensor_tensor(out=mmu, in0=miu_full, in1=miu_full, op=OP.mult)
                    nc.vector.tensor_scalar(mmu, mmu, c2_t, None, OP.mult)
                    sig_full = tiny.tile([10, CD], f32, tag="csg")
                    nc.vector.tensor_scalar(sig_full, m2n, EPS, None, OP.add)
                    nc.vector.tensor_tensor(out=sig_full, in0=sig_full, in1=mmu, op=OP.subtract)

                    miu_diag = tiny.tile([1, CD], f32, tag="dgm")
                    sig_diag = tiny.tile([1, CD], f32, tag="dgs")
                    if it == 0:
                        # r0 is c-uniform: every row of the cross matrix is
                        # the diagonal -> plain same-partition copies
                        nc.vector.tensor_copy(miu_diag, miu_full[0:1, :])
                        nc.vector.tensor_copy(sig_diag, sig_full[0:1, :])
                    else:
                        # diag extraction via DRAM round-trip
                        scr = nc.dram_tensor(f"scr_{s}_{it}", [4096], f32, kind="Internal")[:]
                        nc.gpsimd.dma_start(out=scr[0:1600], in_=miu_full)
                        nc.gpsimd.dma_start(out=scr[1600:3200], in_=sig_full)
                        diag_miu_ap = bass.AP(
                            tensor=scr.tensor,
                            offset=scr.offset,
                            ap=[[0, 1], [176, 10], [1, 16]],
                        )
                        diag_sig_ap = bass.AP(
                            tensor=scr.tensor,
                            offset=scr.offset + 1600,
                            ap=[[0, 1], [176, 10], [1, 16]],
                        )
                        nc.gpsimd.dma_start(out=miu_diag, in_=diag_miu_ap)
                        nc.gpsimd.dma_start(out=sig_diag, in_=diag_sig_ap)

                    # act_out for next iter
                    if it == 0:
                        nc.vector.memset(act_b, 1.0 / C)
                    else:
                        nc.gpsimd.dma_start(out=scr[3200:3210], in_=Rse)
                        Rf = tiny.tile([1, C], f32, tag="pRf")
                        nc.gpsimd.dma_start(out=Rf, in_=scr[3200:3210])
                        mx = tiny.tile([1, 1], f32, tag="pmx")
                        nc.vector.tensor_reduce(mx, Rf, axis=AX.X, op=OP.max)
                        sh = tiny.tile([1, C], f32, tag="psh")
                        nc.vector.tensor_scalar(sh, Rf, mx, None, OP.subtract)
                        ex = tiny.tile([1, C], f32, tag="pex")
                        nc.scalar.activation(out=ex, in_=sh, func=ACTF.Exp)
                        sm = tiny.tile([1, 1], f32, tag="psm")
                        nc.vector.tensor_reduce(sm, ex, axis=AX.X, op=OP.add)
                        smr = tiny.tile([1, 1], f32, tag="psr")
                        nc.vector.reciprocal(smr, sm)
                        ao = tiny.tile([1, C], f32, tag="pao")
                        nc.vector.tensor_scalar(ao, ex, smr, None, OP.mult)
                        nc.gpsimd.partition_broadcast(act_b, ao)
                else:
                    # final: extract diag of miu_full -> output
                    scr = nc.dram_tensor(f"scr_{s}_{it}", [4096], f32, kind="Internal")[:]
                    nc.gpsimd.dma_start(out=scr[0:1600], in_=miu_full)
                    out_diag = bass.AP(
                        tensor=scr.tensor,
                        offset=scr.offset,
                        ap=[[0, 1], [176, 10], [1, 16]],
                    )
                    fin = tiny.tile([1, CD], f32, tag="dgm")
                    nc.gpsimd.dma_start(out=fin, in_=out_diag)
                    nc.gpsimd.dma_start(
                        out=out_d[s].rearrange("c d -> (c d)"), in_=fin
                    )

    nc.compile()
    return nc


def kernel(votes, activation, beta_v, beta_a):
    from concourse.bass_utils import run_bass_kernel_spmd

    if "nc" not in _CACHE:
        _CACHE["nc"] = _build()
    nc = _CACHE["nc"]

    votes = np.ascontiguousarray(votes, dtype=np.float32)
    activation = np.ascontiguousarray(activation, dtype=np.float32)
    in_maps = [
        {
            "votes": votes[i * NB : (i + 1) * NB],
            "activation": activation[i * NB : (i + 1) * NB],
        }
        for i in range(NCORES)
    ]
    res = run_bass_kernel_spmd(nc, in_maps, core_ids=list(range(NCORES)))
    out = np.concatenate([res.results[i]["miu_out"] for i in range(NCORES)], axis=0)
    return out.reshape(B, 1, C, D).astype(np.float32)


if __name__ == "__main__":
    _build()
    print("build OK")


# revision 35
# speedup vs baseline: 1.5301x; 1.0658x over previous
"""EM-routing (DigitCaps) kernel for 8 trn2 NeuronCores.

Sharding: data-parallel over batch B=16 -> 2 samples per core, no collectives.

Per-core algorithm (N=16384, C=10, D=16, 3 EM iterations, output = miu):
  - votes stored in SBUF f32 N-layout vv[p, n2, 0:160]=v, [160:320]=v^2,
    [320]=1.0, with n = p*128 + n2  (one sample at a time: 160.5KB/partition).
  - All n-reductions (M1=sum_n r*v, M2=sum_n r*v^2, R=sum_n r) are PE matmuls
    with the per-iteration routing weights r as the stationary operand
    (plain fp32 matmuls: exact, 4 cyc/row). The matmul
    produces the full 10x160 cross matrix; the needed diagonal c-blocks are
    extracted via a DRAM round-trip (diag AP [[176,10],[1,16]]).
  - sigma via moments: sigma = M2/R - miu^2*(2-S) + eps, S = R/(R+eps).
  - log-p d-contraction: q[n,c] = A2_c + sum_d v*g - sum_d v^2*s2 with
    g=miu/sigma, s2=1/(2*sigma) broadcast along partitions; computed by DVE
    tensor ops + segmented reduces.
  - The reference's per-n max-shift is replaced by the constant upper bound
    max_cd(h), h=-0.5*log(sigma) (difference enters only through eps, ~1e-21).
"""

import numpy as np

B, N, C, D = 16, 16384, 10, 16
CD = C * D
NB = 2          # samples per core
NCORES = 8
P = 128         # partitions
NCH = N // P    # 128 chunks; n = p*128 + n2
EPS = 1e-9
LN10 = float(np.log(10.0))
SLAB = 4        # chunks per z-slab
NSLAB = NCH // SLAB

_CACHE = {}


def _build():
    from contextlib import ExitStack
    import concourse.bass as bass
    import concourse.bacc as bacc
    import concourse.mybir as mybir
    import concourse.tile as tile
    import concourse.bass_isa as bass_isa

    f32 = mybir.dt.float32
    f32r = mybir.dt.float32r
    AX = mybir.AxisListType
    OP = mybir.AluOpType
    ACTF = mybir.ActivationFunctionType

    nc = bacc.Bacc("TRN2")
    votes_d = nc.dram_tensor("votes", [NB, N, C, D], f32, kind="ExternalInput")
    act_d = nc.dram_tensor("activation", [NB, N, 1], f32, kind="ExternalInput")
    out_d = nc.dram_tensor("miu_out", [NB, C, D], f32, kind="ExternalOutput")

    def bfree(t_ap, nrep):
        # read-broadcast an SBUF AP along a new middle free dim (step 0)
        return bass.AP(
            tensor=t_ap.tensor,
            offset=t_ap.offset,
            ap=[t_ap.ap[0], [0, nrep]] + list(t_ap.ap[1:]),
        )

    with ExitStack() as ctx:
        tc = ctx.enter_context(tile.TileContext(nc))
        big = ctx.enter_context(tc.tile_pool(name="big", bufs=1))
        zpool = ctx.enter_context(tc.tile_pool(name="z", bufs=3))
        qpool = ctx.enter_context(tc.tile_pool(name="q", bufs=1))
        rpool = ctx.enter_context(tc.tile_pool(name="r", bufs=1))
        small = ctx.enter_context(tc.tile_pool(name="small", bufs=1))
        tiny = ctx.enter_context(tc.tile_pool(name="tiny", bufs=1))
        psum = ctx.enter_context(tc.tile_pool(name="psum", bufs=2, space="PSUM"))

        # persistent across the whole kernel
        vv = big.tile([P, NCH, 321], f32)          # [v | v^2 | ones]
        a_t = big.tile([P, NCH], f32)              # activation
        act_b = big.tile([P, C], f32)              # act_out bcast (free c)

        for s in range(NB):
            # ---------------- load sample s ----------------
            if s > 0:
                nc.all_engine_barrier()
            nc.gpsimd.dma_start(
                out=a_t[:, :], in_=act_d[s].rearrange("(p n2) one -> p (n2 one)", p=P)
            )
            vsrc = votes_d[s].rearrange("(p n2) c d -> p n2 (c d)", p=P)
            for g in range(16):
                sl = slice(g * 8, (g + 1) * 8)
                nc.gpsimd.dma_start(out=vv[:, sl, 0:160], in_=vsrc[:, sl, :])
                nc.scalar.activation(
                    out=vv[:, sl, 161:321], in_=vv[:, sl, 0:160], func=ACTF.Square
                )
                nc.scalar.activation(
                    out=vv[:, sl, 160:161], in_=vv[:, sl, 0:1],
                    func=ACTF.Copy, bias=1.0, scale=0.0,
                )

            r_t = rpool.tile([P, NCH, C], f32, tag="r")
            q_t = qpool.tile([P, NCH, C], f32)

            miu_diag = None
            for it in range(3):
                # ---------------- r computation ----------------
                if it == 0:
                    t0 = small.tile([P, NCH], f32, tag="sm0")
                    nc.vector.tensor_scalar_add(t0, a_t, EPS)
                    t1 = small.tile([P, NCH], f32, tag="sm1")
                    nc.vector.reciprocal(t1, t0)
                    t2 = small.tile([P, NCH], f32, tag="sm2")
                    nc.vector.tensor_tensor(out=t2, in0=a_t, in1=t1, op=OP.mult)
                    # r0 = a/(C*(a+eps)) broadcast over c (on ACT so the
                    # first matmul's waits collapse to {ACT, DMA})
                    nc.scalar.activation(
                        r_t[:, :, :],
                        bass.AP(
                            tensor=t2.tensor,
                            offset=t2.offset,
                            ap=[t2.ap[0], t2.ap[1], [0, C]],
                        ),
                        func=ACTF.Copy,
                        scale=1.0 / C,
                    )
                    # canary: dummy matmul consuming an ACT-written tile so the
                    # real matmuls' PE-side waits collapse to one semaphore
                    cn = tiny.tile([1, 1], f32, tag="cn")
                    nc.scalar.activation(out=cn, in_=a_t[0:1, 0:1], func=ACTF.Copy)
                    psc = psum.tile([1, 1], f32, tag="psc")
                    nc.tensor.matmul(psc, cn, cn, start=True, stop=True)
                    mm = psum.tile([10, 321], f32, tag="mm")
                    for n2 in range(NCH):
                        nc.tensor.matmul(
                            mm,
                            r_t[:, n2, :],
                            vv[:, n2, :],
                            start=(n2 == 0),
                            stop=(n2 == NCH - 1),
                        )
                else:
                    # params from miu_diag/sig_diag (single-partition [1,160])
                    inv_sig = tiny.tile([1, CD], f32, tag="pinv")
                    nc.vector.reciprocal(inv_sig, sig_diag)
                    g_f = tiny.tile([1, CD], f32, tag="pg")
                    nc.vector.tensor_tensor(out=g_f, in0=miu_diag, in1=inv_sig, op=OP.mult)
                    s2_f = tiny.tile([1, CD], f32, tag="ps2")
                    nc.vector.tensor_scalar_mul(s2_f, inv_sig, 0.5)
                    ls_f = tiny.tile([1, CD], f32, tag="pls")
                    nc.scalar.activation(out=ls_f, in_=sig_diag, func=ACTF.Ln)
                    # u = log(sig) + miu^2/sig ; A = -0.5 * sum_d u per c
                    w_f = tiny.tile([1, CD], f32, tag="pw")
                    nc.vector.tensor_tensor(out=w_f, in0=miu_diag, in1=g_f, op=OP.mult)
                    u_f = tiny.tile([1, CD], f32, tag="pu")
                    nc.vector.tensor_tensor(out=u_f, in0=ls_f, in1=w_f, op=OP.add)
                    Ac = tiny.tile([1, C], f32, tag="pAc")
                    nc.vector.tensor_reduce(
                        Ac, u_f.rearrange("one (c d) -> one c d", c=C), axis=AX.X, op=OP.add
                    )
                    # m' = max_cd h = -0.5*min_cd log(sig)
                    lmin = tiny.tile([1, 1], f32, tag="plm")
                    nc.vector.tensor_reduce(lmin, ls_f, axis=AX.X, op=OP.min)
                    # A2_c = -0.5*Ac + D*(ln10 + 0.5*lmin)
                    bias_t = tiny.tile([1, 1], f32, tag="pbi")
                    nc.vector.tensor_scalar(
                        bias_t, lmin, 0.5 * float(D), float(D) * LN10, OP.mult, OP.add
                    )
                    A2 = tiny.tile([1, C], f32, tag="pA2")
                    nc.vector.tensor_scalar(A2, Ac, -0.5, bias_t, OP.mult, OP.add)
                    A2_128 = small.tile([P, C], f32, tag="a2b")
                    nc.gpsimd.partition_broadcast(A2_128, A2)
                    g_b = small.tile([P, CD], f32, tag="gb")
                    nc.gpsimd.partition_broadcast(g_b, g_f)
                    s2_b = small.tile([P, CD], f32, tag="s2b")
                    nc.gpsimd.partition_broadcast(s2_b, s2_f)

                    # ---------------- q volume + r chain + matmuls, blocked
                    # over n so PE matmuls of block k overlap the DVE chain
                    # of block k+1 ----------------------------------------
                    g_b3 = g_b.rearrange("p (c d) -> p c d", c=C)
                    s2_b3 = s2_b.rearrange("p (c d) -> p c d", c=C)
                    FW = 321 if it < 2 else 161
                    mm = psum.tile([10, 321], f32, tag="mm")
                    sc = small.tile([P, NCH], f32, tag="sm0")
                    d1 = small.tile([P, NCH], f32, tag="sm1")
                    d2 = small.tile([P, NCH], f32, tag="sm2")
                    fac1 = small.tile([P, NCH], f32, tag="sm3")
                    sc2 = small.tile([P, NCH], f32, tag="sm4")
                    d3 = small.tile([P, NCH], f32, tag="sm5")
                    d4 = small.tile([P, NCH], f32, tag="sm6")
                    NBLK = 16
                    BCH = NCH // NBLK
                    for blk in range(NBLK):
                        for sb in range(BCH // SLAB):
                            sl = slice(
                                blk * BCH + sb * SLAB, blk * BCH + (sb + 1) * SLAB
                            )
                            z1 = zpool.tile([P, SLAB, C, D], f32, tag="z")
                            nc.vector.tensor_tensor(
                                out=z1,
                                in0=vv[:, sl, 0:160].rearrange(
                                    "p s (c d) -> p s c d", c=C
                                ),
                                in1=bfree(g_b3, SLAB),
                                op=OP.mult,
                            )
                            z2 = zpool.tile([P, SLAB, C, D], f32, tag="z")
                            nc.gpsimd.tensor_tensor(
                                out=z2,
                                in0=vv[:, sl, 161:321].rearrange(
                                    "p s (c d) -> p s c d", c=C
                                ),
                                in1=bfree(s2_b3, SLAB),
                                op=OP.mult,
                            )
                            q1 = zpool.tile([P, SLAB, C], f32, tag="qs")
                            nc.vector.tensor_reduce(q1, z1, axis=AX.X, op=OP.add)
                            q2 = zpool.tile([P, SLAB, C], f32, tag="qs")
                            nc.vector.tensor_reduce(q2, z2, axis=AX.X, op=OP.add)
                            nc.gpsimd.tensor_tensor(
                                out=q_t[:, sl, :], in0=q1, in1=q2, op=OP.subtract
                            )
                        bs = slice(blk * BCH, (blk + 1) * BCH)
                        qb = q_t[:, bs, :]
                        nc.vector.tensor_tensor(
                            out=qb, in0=qb, in1=bfree(A2_128, BCH), op=OP.add
                        )
                        nc.scalar.activation(out=qb, in_=qb, func=ACTF.Exp)
                        nc.vector.tensor_tensor(
                            out=qb, in0=qb, in1=bfree(act_b, BCH), op=OP.mult
                        )
                        nc.vector.tensor_reduce(sc[:, bs], qb, axis=AX.X, op=OP.add)
                        nc.vector.tensor_scalar_add(d1[:, bs], sc[:, bs], EPS)
                        nc.vector.reciprocal(d2[:, bs], d1[:, bs])
                        nc.vector.tensor_tensor(
                            out=fac1[:, bs], in0=d2[:, bs], in1=a_t[:, bs], op=OP.mult
                        )
                        f1s = fac1[:, bs]
                        nc.vector.tensor_tensor(
                            out=qb,
                            in0=qb,
                            in1=bass.AP(
                                tensor=f1s.tensor,
                                offset=f1s.offset,
                                ap=[f1s.ap[0], f1s.ap[1], [0, C]],
                            ),
                            op=OP.mult,
                        )
                        nc.vector.tensor_reduce(sc2[:, bs], qb, axis=AX.X, op=OP.add)
                        nc.vector.tensor_scalar_add(d3[:, bs], sc2[:, bs], EPS)
                        nc.vector.reciprocal(d4[:, bs], d3[:, bs])
                        d4s = d4[:, bs]
                        nc.vector.tensor_tensor(
                            out=r_t[:, bs, :],
                            in0=qb,
                            in1=bass.AP(
                                tensor=d4s.tensor,
                                offset=d4s.offset,
                                ap=[d4s.ap[0], d4s.ap[1], [0, C]],
                            ),
                            op=OP.mult,
                        )
                        for n2 in range(blk * BCH, (blk + 1) * BCH):
                            nc.tensor.matmul(
                                mm[:, 0:FW],
                                r_t[:, n2, :],
                                vv[:, n2, 0:FW],
                                start=(n2 == 0),
                                stop=(n2 == NCH - 1),
                            )


                # ---------------- stats (full 10x160 cross matrices) --------
                Rse = tiny.tile([10, 1], f32, tag="cRs")
                nc.vector.tensor_scalar_add(Rse, mm[:, 160:161], EPS)
                invR = tiny.tile([10, 1], f32, tag="cIR")
                nc.vector.reciprocal(invR, Rse)
                miu_full = small.tile([10, CD], f32, tag="miuf")
                nc.vector.tensor_scalar(miu_full, mm[:, 0:160], invR, None, OP.mult)

                if it < 2:
                    m2n = tiny.tile([10, CD], f32, tag="cm2")
                    nc.vector.tensor_scalar(m2n, mm[:, 161:321], invR, None, OP.mult)
                    S_t = tiny.tile([10, 1], f32, tag="cS")
                    nc.vector.tensor_scalar(S_t, mm[:, 160:161], invR, None, OP.mult)
                    c2_t = tiny.tile([10, 1], f32, tag="cc2")
                    nc.vector.tensor_scalar(c2_t, S_t, -1.0, 2.0, OP.mult, OP.add)
                    mmu = tiny.tile([10, CD], f32, tag="cmu")
                    nc.vector.tensor_tensor(out=mmu, in0=miu_full, in1=miu_full, op=OP.mult)
                    nc.vector.tensor_scalar(mmu, mmu, c2_t, None, OP.mult)
                    sig_full = tiny.tile([10, CD], f32, tag="csg")
                    nc.vector.tensor_scalar(sig_full, m2n, EPS, None, OP.add)
                    nc.vector.tensor_tensor(out=sig_full, in0=sig_full, in1=mmu, op=OP.subtract)

                    miu_diag = tiny.tile([1, CD], f32, tag="dgm")
                    sig_diag = tiny.tile([1, CD], f32, tag="dgs")
                    if it == 0:
                        # r0 is c-uniform: every row of the cross matrix is
                        # the diagonal -> plain same-partition copies
                        nc.vector.tensor_copy(miu_diag, miu_full[0:1, :])
                        nc.vector.tensor_copy(sig_diag, sig_full[0:1, :])
                    else:
                        # diag extraction via DRAM round-trip
                        scr = nc.dram_tensor(f"scr_{s}_{it}", [4096], f32, kind="Internal")[:]
                        nc.gpsimd.dma_start(out=scr[0:1600], in_=miu_full)
                        nc.gpsimd.dma_start(out=scr[1600:3200], in_=sig_full)
                        diag_miu_ap = bass.AP(
                            tensor=scr.tensor,
                            offset=scr.offset,
                            ap=[[0, 1], [176, 10], [1, 16]],
                        )
                        diag_sig_ap = bass.AP(
                            tensor=scr.tensor,
                            offset=scr.offset + 1600,
                            ap=[[0, 1], [176, 10], [1, 16]],
                        )
                        nc.gpsimd.dma_start(out=miu_diag, in_=diag_miu_ap)
                        nc.gpsimd.dma_start(out=sig_diag, in_=diag_sig_ap)

                    # act_out for next iter
                    if it == 0:
                        nc.vector.memset(act_b, 1.0 / C)
                    else:
                        nc.gpsimd.dma_start(out=scr[3200:3210], in_=Rse)
                        Rf = tiny.tile([1, C], f32, tag="pRf")
                        nc.gpsimd.dma_start(out=Rf, in_=scr[3200:3210])
                        mx = tiny.tile([1, 1], f32, tag="pmx")
                        nc.vector.tensor_reduce(mx, Rf, axis=AX.X, op=OP.max)
                        sh = tiny.tile([1, C], f32, tag="psh")
                        nc.vector.tensor_scalar(sh, Rf, mx, None, OP.subtract)
                        ex = tiny.tile([1, C], f32, tag="pex")
                        nc.scalar.activation(out=ex, in_=sh, func=ACTF.Exp)
                        sm = tiny.tile([1, 1], f32, tag="psm")
                        nc.vector.tensor_reduce(sm, ex, axis=AX.X, op=OP.add)
                        smr = tiny.tile([1, 1], f32, tag="psr")
                        nc.vector.reciprocal(smr, sm)
                        ao = tiny.tile([1, C], f32, tag="pao")
                        nc.vector.tensor_scalar(ao, ex, smr, None, OP.mult)
                        nc.gpsimd.partition_broadcast(act_b, ao)
                else:
                    # final: extract diag of miu_full -> output
                    scr = nc.dram_tensor(f"scr_{s}_{it}", [4096], f32, kind="Internal")[:]
                    nc.gpsimd.dma_start(out=scr[0:1600], in_=miu_full)
                    out_diag = bass.AP(
                        tensor=scr.tensor,
                        offset=scr.offset,
                        ap=[[0, 1], [176, 10], [1, 16]],
                    )
                    fin = tiny.tile([1, CD], f32, tag="dgm")
                    nc.gpsimd.dma_start(out=fin, in_=out_diag)
                    nc.gpsimd.dma_start(
                        out=out_d[s].rearrange("c d -> (c d)"), in_=fin
                    )

    nc.compile()
    return nc


def kernel(votes, activation, beta_v, beta_a):
    from concourse.bass_utils import run_bass_kernel_spmd

    if "nc" not in _CACHE:
        _CACHE["nc"] = _build()
    nc = _CACHE["nc"]

    votes = np.ascontiguousarray(votes, dtype=np.float32)
    activation = np.ascontiguousarray(activation, dtype=np.float32)
    in_maps = [
        {
            "votes": votes[i * NB : (i + 1) * NB],
            "activation": activation[i * NB : (i + 1) * NB],
        }
        for i in range(NCORES)
    ]
    res = run_bass_kernel_spmd(nc, in_maps, core_ids=list(range(NCORES)))
    out = np.concatenate([res.results[i]["miu_out"] for i in range(NCORES)], axis=0)
    return out.reshape(B, 1, C, D).astype(np.float32)


if __name__ == "__main__":
    _build()
    print("build OK")
